# revision 1
# baseline (speedup 1.0000x reference)
"""Trainium2 Bass kernel for nn_ContactMapHead.

Reference computation (per sample b):
  m = (attention_mask==1) & (special_tokens_mask==0)     # valid positions
  S[t,s] = h_t^T W h_s + bias                            # bilinear scores
  out[b, :] = strict-upper-triangle (row-major) of S restricted to the
              compacted valid positions, scattered into a MAX_PAIRS buffer
              (rest zeros).

Strategy (data-parallel over batch across 8 NeuronCores):
  - Host: compute valid index lists, gather + transpose each sample's valid
    rows into a packed hT buffer (H x 4*NP per core, NP = max valid count,
    zero-padded). This is index bookkeeping on tiny int masks; all tensor
    FLOPs stay on device.
  - Device (per core, 4 samples): u^T = W^T @ hT (fp32r on the PE at full
    rate), then per-sample S = u^T.T @ hT (fp32). Ships the per-sample
    S matrices (NP x NP) back.
  - Host: out[b, :npairs] = S[triu] + bias; remainder stays zero.

The NEFF is compiled for the actual NP of the given inputs (cached per
shape), so the kernel adapts to any mask density.
"""

import os

import numpy as np

import concourse.bass as bass  # noqa: F401  (bass types used via tile/bacc)
import concourse.tile as tile
from concourse import bacc, mybir
from concourse.bass_utils import run_bass_kernel_spmd

MAX_PAIRS = 130816
N_CORES = 8
H = 1024
KT = H // 128  # contraction k-tiles
SPB = 4  # samples per core (B=32 over 8 cores)

# Compiled-module cache: (NP, spg, fp32r) -> Bacc
_nc_cache: dict = {}


def _build(NP: int, spg: int, use_fp32r: bool):
    """Build + compile the per-core NEFF for padded sample width NP.

    spg = samples per u-step column group (group width spg*NP <= 512).
    """
    Wt = SPB * NP  # packed width per core
    f32 = mybir.dt.float32
    f32r = mybir.dt.float32r

    nc = bacc.Bacc(
        "TRN2",
        target_bir_lowering=False,
        debug=False,
        enable_asserts=False,
        num_devices=N_CORES,
    )
    hT_d = nc.dram_tensor("hT", [H, Wt], f32, kind="ExternalInput").ap()
    W_d = nc.dram_tensor("W", [H, H], f32, kind="ExternalInput").ap()
    S_d = nc.dram_tensor("S_out", [SPB, NP, NP], f32, kind="ExternalOutput").ap()

    with tile.TileContext(nc) as tc:
        with (
            tc.tile_pool(name="wpool", bufs=1) as wpool,
            tc.tile_pool(name="hpool", bufs=1) as hpool,
            tc.tile_pool(name="upool", bufs=1) as upool,
            tc.tile_pool(name="scratch", bufs=2) as scratch,
            tc.tile_pool(name="spool", bufs=3) as spool,
            tc.tile_pool(name="ps", bufs=1, space="PSUM") as ps,
        ):
            # ---- load W k-tiles; cast to fp32r for the u-step ----
            W_mm = []
            for kk in range(KT):
                if use_fp32r:
                    wtmp = scratch.tile([128, H], f32, tag="wtmp")
                    nc.sync.dma_start(wtmp[:], W_d[128 * kk : 128 * (kk + 1), :])
                    wr = wpool.tile([128, H], f32r, name=f"wr{kk}")
                    nc.vector.tensor_copy(wr[:], wtmp[:])
                else:
                    wr = wpool.tile([128, H], f32, name=f"wr{kk}")
                    nc.sync.dma_start(wr[:], W_d[128 * kk : 128 * (kk + 1), :])
                W_mm.append(wr)

            # ---- load hT k-tiles (fp32 kept for the S-step; fp32r for u) ----
            h32 = []
            h_mm = []
            for kk in range(KT):
                h = hpool.tile([128, Wt], f32, name=f"h32_{kk}")
                nc.sync.dma_start(h[:], hT_d[128 * kk : 128 * (kk + 1), :])
                h32.append(h)
                if use_fp32r:
                    hr = hpool.tile([128, Wt], f32r, name=f"hr{kk}")
                    nc.vector.tensor_copy(hr[:], h[:])
                    h_mm.append(hr)
                else:
                    h_mm.append(h)

            # ---- u-step: u^T[k', c] = sum_k W[k, k'] * hT[k, c] ----
            u32 = [
                upool.tile([128, Wt], f32, name=f"u32_{kk}") for kk in range(KT)
            ]
            gw = spg * NP
            for c0 in range(0, Wt, gw):
                c1 = min(c0 + gw, Wt)
                for kkp in range(KT):
                    p = ps.tile([128, c1 - c0], f32, tag="u_ps", bufs=3)
                    for kk in range(KT):
                        nc.tensor.matmul(
                            p[:],
                            W_mm[kk][:, 128 * kkp : 128 * (kkp + 1)],
                            h_mm[kk][:, c0:c1],
                            start=(kk == 0),
                            stop=(kk == KT - 1),
                        )
                    nc.vector.tensor_copy(u32[kkp][:, c0:c1], p[:])

            # ---- S-step (fp32): S[t, s] = sum_k u^T[k, t] * hT[k, s] ----
            # Only rows t and cols s>t matter; compute t-chunks of 128 rows,
            # cols [t0+1, NP).
            for j in range(SPB):
                o = j * NP
                for t0 in range(0, NP, 128):
                    tsz = min(128, NP - t0)
                    nsz = NP - t0 - 1
                    if nsz <= 0:
                        continue
                    pS = ps.tile([tsz, nsz], f32, tag="s_ps", bufs=2)
                    for kk in range(KT):
                        nc.tensor.matmul(
                            pS[:],
                            u32[kk][:, o + t0 : o + t0 + tsz],
                            h32[kk][:, o + t0 + 1 : o + NP],
                            start=(kk == 0),
                            stop=(kk == KT - 1),
                        )
                    sS = spool.tile([tsz, nsz], f32, tag="s_sb")
                    nc.any.tensor_copy(sS[:], pS[:])
                    nc.sync.dma_start(
                        S_d[j, t0 : t0 + tsz, t0 + 1 : NP], sS[:]
                    )

    nc.compile()
    return nc


def _get_nc(NP: int, spg: int, use_fp32r: bool):
    key = (NP, spg, use_fp32r)
    if key not in _nc_cache:
        _nc_cache[key] = _build(NP, spg, use_fp32r)
    return _nc_cache[key]


def kernel(hidden_states, attention_mask, special_tokens_mask, W, b):
    hs = np.asarray(hidden_states, dtype=np.float32)
    am = np.asarray(attention_mask)
    sm = np.asarray(special_tokens_mask)
    Wm = np.ascontiguousarray(np.asarray(W, dtype=np.float32))
    bias = float(np.asarray(b).reshape(-1)[0])

    B, L, Hdim = hs.shape
    assert Hdim == H and B == N_CORES * SPB, (B, L, Hdim)

    m = (am == 1) & (sm == 0)
    idxs = [np.nonzero(m[i])[0] for i in range(B)]
    ns = [len(ix) for ix in idxs]
    NP = int(max(ns))

    out = np.zeros((B, MAX_PAIRS), np.float32)
    if NP < 2:
        return out

    # u-step group width spg*NP: biggest that fits a 512-wide PSUM bank.
    spg = max(s for s in (4, 2, 1) if s == 1 or s * NP <= 512)

    use_fp32r = os.environ.get("CONTACT_KERNEL_FP32", "0") != "1"
    nc = _get_nc(NP, spg, use_fp32r)

    in_maps = []
    for c in range(N_CORES):
        hT = np.zeros((H, SPB * NP), np.float32)
        for j in range(SPB):
            bi = c * SPB + j
            ix = idxs[bi]
            if len(ix):
                hT[:, j * NP : j * NP + len(ix)] = hs[bi].T[:, ix]
        in_maps.append({"hT": hT, "W": Wm})

    res = run_bass_kernel_spmd(nc, in_maps, core_ids=list(range(N_CORES)))

    tri_cache: dict = {}
    for c in range(N_CORES):
        S = res.results[c]["S_out"]  # (SPB, NP, NP)
        for j in range(SPB):
            bi = c * SPB + j
            n = ns[bi]
            if n < 2:
                continue
            if n not in tri_cache:
                tri_cache[n] = np.triu_indices(n, 1)
            iu, ju = tri_cache[n]
            out[bi, : iu.size] = S[j][iu, ju] + bias
    return out


# revision 18
# speedup vs baseline: 1.4767x; 1.4767x over previous
"""Trainium2 Bass kernel for nn_ContactMapHead.

Reference computation (per sample b):
  m = (attention_mask==1) & (special_tokens_mask==0)     # valid positions
  S[t,s] = h_t^T W h_s + bias                            # bilinear scores
  out[b, :] = strict-upper-triangle (row-major) of S restricted to the
              compacted valid positions, scattered into a MAX_PAIRS buffer
              (rest zeros).

Strategy (data-parallel over batch across 8 NeuronCores):
  - Host: compute valid index lists; gather + transpose each sample's valid
    rows into a packed hT buffer (H x Wt per core, zero padded to per-slot
    widths shared across cores so one SPMD NEFF serves all 8 cores). Index
    bookkeeping only — all tensor FLOPs run on device.
  - Device (per core, 4 samples): u^T = W^T @ hT, then per-sample
    S = u^T.T @ hT. Ships the per-sample S matrices back.
  - Host: out[b, :npairs] = S[triu] + bias; remainder stays zero.

Precision modes (CONTACT_KERNEL_PRECISION):
  fast:     u-step fp16, S-step fp16   (~5e-4 rel-absmax)
  balanced: u-step fp16, S-step fp32   (~2.5e-4)  [default]
  accurate: u-step fp32r, S-step fp32  (~1.3e-4)
  exact:    everything fp32            (~1e-6, ~3x slower)

Samples are assigned to (core, slot) by descending valid-count so per-slot
padded widths (max over cores) waste little compute. The NEFF is compiled
for the actual widths of the given inputs (cached per shape).
"""

import os

import numpy as np

import concourse.bass as bass  # noqa: F401
import concourse.tile as tile
from concourse import bacc, mybir
from concourse.bass_utils import run_bass_kernel_spmd

MAX_PAIRS = 130816
N_CORES = 8
H = 1024
KT = H // 128  # k-tiles along the contraction dim
SPB = 4  # samples (slots) per core

_nc_cache: dict = {}


def _make_groups(NPs, u_dt):
    """Partition slots into column groups, each of total width <= 512 (PSUM
    bank / moving-operand limit), minimizing modeled u-step cost. fp32r runs
    4 cyc/row below width 256; fp16/fp32 cost is width-proportional. Brute
    forces all set partitions of the (at most 4) slots."""
    n = len(NPs)

    def partitions(items):
        if not items:
            yield []
            return
        first, rest = items[0], items[1:]
        for part in partitions(rest):
            for i in range(len(part)):
                yield part[:i] + [[first] + part[i]] + part[i + 1 :]
            yield [[first]] + part

    def gcost(w):
        if u_dt == "fp32":
            return w * 4.0
        if u_dt == "fp32r":
            return w * (4.0 if w < 256 else 1.0)
        # fp16: per chain-step cost ~ max(weight-load ~107ns, stream w/2.4GHz)
        # (LDWEIGHTS pipelines against the previous matmul on HW)
        return max(107.0, w * 0.4167) / 0.4167

    best, best_cost = None, None
    for part in partitions(list(range(n))):
        cost = 0.0
        ok = True
        for g in part:
            w = sum(NPs[s] for s in g)
            if w > 512:
                ok = False
                break
            cost += gcost(w)
        if ok and (best_cost is None or (cost, len(part)) < best_cost):
            best_cost = (cost, len(part))
            best = part
    assert best is not None
    return [sorted(g) for g in best]


def _build(NPs_layout, group_sizes, u_dt, s16, loop_iters=None):
    """Build + compile the per-core NEFF.

    NPs_layout: per-slot padded widths in packed column order (groups are
    contiguous runs of slots given by group_sizes).
    loop_iters: benchmark mode — repeat the whole body N times on-device so
    wall-clock deltas between two iteration counts give per-iteration HW
    time through the axon tunnel (no NTFF profiling available there)."""
    f32 = mybir.dt.float32
    mm_dt = {"fp16": mybir.dt.float16, "fp32r": mybir.dt.float32r, "fp32": f32}[u_dt]
    s_dt = mybir.dt.float16 if s16 else f32
    Wt = sum(NPs_layout)
    offs = np.concatenate([[0], np.cumsum(NPs_layout)]).astype(int)
    granges = []
    s = 0
    for gs in group_sizes:
        granges.append((int(offs[s]), int(offs[s + gs])))
        s += gs
    n_groups = len(granges)

    nc = bacc.Bacc(
        "TRN2",
        target_bir_lowering=False,
        debug=False,
        enable_asserts=False,
        num_devices=N_CORES,
    )
    hT_d = nc.dram_tensor("hT", [H, Wt], f32, kind="ExternalInput").ap()
    W_d = nc.dram_tensor("W", [H, H], mm_dt, kind="ExternalInput").ap()
    S_d = [
        nc.dram_tensor(f"S{j}", [NPs_layout[j], NPs_layout[j]], f32, kind="ExternalOutput").ap()
        for j in range(SPB)
    ]

    out_engines = [nc.sync, nc.scalar]

    with tile.TileContext(nc) as tc:
        with (
            tc.tile_pool(name="wpool", bufs=1) as wpool,
            tc.tile_pool(name="hpool", bufs=1) as hpool,
            tc.tile_pool(name="upool", bufs=1) as upool,
            tc.tile_pool(name="spool", bufs=4) as spool,
            tc.tile_pool(name="ps", bufs=8, space="PSUM") as ps,
        ):
            # ---- PE warmup: keep HAM busy while input DMAs land ----
            # (plain fp32 warmup tiles: memset of float32r is illegal ISA)
            wz_a = wpool.tile([128, 16], f32, name="wz_a")
            nc.gpsimd.memset(wz_a[:], 0.0)
            wz_b = wpool.tile([128, 64], f32, name="wz_b")
            nc.gpsimd.memset(wz_b[:], 0.0)
            pwarm = ps.tile([16, 64], f32, tag="ps", name="pwarm")
            for _ in range(32):
                nc.tensor.matmul(pwarm[:], wz_a[:], wz_b[:], start=True, stop=True)
            # warm the ACT Copy table so the first real scalar.copy is cheap
            wz_c = wpool.tile([16, 64], f32, name="wz_c")
            nc.scalar.copy(wz_c[:], pwarm[:])

            # ---- W k-tiles (pre-rounded on host; SP-triggered DMA) ----
            W_mm = []
            for kk in range(KT):
                wr = wpool.tile([128, H], mm_dt, name=f"wr{kk}")
                nc.sync.dma_start(wr[:], W_d[128 * kk : 128 * (kk + 1), :])
                W_mm.append(wr)

            # ---- hT k-tiles (ACT-triggered DMA; DVE cast for the u-step) ----
            h32 = []
            h_mm = []
            for kk in range(KT):
                h = hpool.tile([128, Wt], f32, name=f"h32_{kk}")
                nc.scalar.dma_start(h[:], hT_d[128 * kk : 128 * (kk + 1), :])
                h32.append(h)
                if u_dt == "fp32":
                    h_mm.append(h)
                else:
                    hr = hpool.tile([128, Wt], mm_dt, name=f"hr{kk}")
                    nc.vector.tensor_copy(hr[:], h[:])
                    h_mm.append(hr)

            # S-step operand tiles (fp16 S reads the cast h; fp32 S reads h32)
            h_s = h_mm if s16 else h32

            # ---- u-step: u^T[k',c] = sum_k W[k,k'] hT[k,c] ----
            # kk-outer accumulation so the PE consumes k-tiles as they
            # arrive. Live PSUM chains (n_groups x kkp_wave) bounded by the
            # 8 PSUM banks. For the slower fp32r/fp32 paths (longer DMA
            # window), the contraction is additionally split into kk-waves
            # with SBUF accumulation so every kkp-chain makes progress while
            # early k-tiles arrive.
            u_sb = [upool.tile([128, Wt], s_dt, name=f"u_{kk}") for kk in range(KT)]
            kkp_wave = max(1, 8 // n_groups)
            kk_wave = KT if (n_groups == 1 or u_dt == "fp16") else KT // 2
            for w0 in range(0, KT, kk_wave):
                kks = range(w0, min(w0 + kk_wave, KT))
                for h0 in range(0, KT, kkp_wave):
                    kkps = range(h0, min(h0 + kkp_wave, KT))
                    chains = {}
                    for kk in kks:
                        for gi, (c0, c1) in enumerate(granges):
                            for kkp in kkps:
                                if kk == kks[0]:
                                    chains[(gi, kkp)] = ps.tile(
                                        [128, c1 - c0],
                                        f32,
                                        tag="ps",
                                        name=f"ups{gi}_{kkp}_{w0}",
                                    )
                                nc.tensor.matmul(
                                    chains[(gi, kkp)][:],
                                    W_mm[kk][:, 128 * kkp : 128 * (kkp + 1)],
                                    h_mm[kk][:, c0:c1],
                                    start=(kk == kks[0]),
                                    stop=(kk == kks[-1]),
                                )
                    for gi, (c0, c1) in enumerate(granges):
                        for kkp in kkps:
                            dst = u_sb[kkp][:, c0:c1]
                            src = chains[(gi, kkp)][:]
                            if w0 == 0:
                                if kkp % 2 == 0:
                                    nc.vector.tensor_copy(dst, src)
                                else:
                                    nc.scalar.copy(dst, src)
                            else:
                                # elementwise tensor+tensor add is DVE-only
                                nc.vector.tensor_add(dst, dst, src)

            # ---- S-step: S[t,s] = sum_k' u^T[k',t] hT[k',s] ----
            # Largest chunks first so the kernel tail ends on the smallest
            # chunk's copy + DMA.
            # Chunks stream cols [t0, NPj) — the diagonal column is computed
            # and ignored by the host so the moving width (NPj - t0) stays
            # EVEN (PE matmuls fault on odd moving widths; NPj is even).
            s_chunks = []
            for j in range(SPB):
                NPj = NPs_layout[j]
                for t0 in range(0, NPj, 128):
                    tsz = min(128, NPj - t0)
                    nsz = NPj - t0
                    if nsz > 1:
                        s_chunks.append((tsz * nsz, j, t0, tsz, nsz))
            s_chunks.sort(reverse=True)
            for ci, (_, j, t0, tsz, nsz) in enumerate(s_chunks):
                o = int(offs[j])
                NPj = NPs_layout[j]
                pS = ps.tile([tsz, nsz], f32, tag="ps", name=f"sps{j}_{t0}")
                for kk in range(KT):
                    nc.tensor.matmul(
                        pS[:],
                        u_sb[kk][:, o + t0 : o + t0 + tsz],
                        h_s[kk][:, o + t0 : o + NPj],
                        start=(kk == 0),
                        stop=(kk == KT - 1),
                    )
                sS = spool.tile([tsz, nsz], f32, tag="s_sb")
                if ci % 2 == 0:
                    nc.vector.tensor_copy(sS[:], pS[:])
                else:
                    nc.scalar.copy(sS[:], pS[:])
                out_engines[ci % len(out_engines)].dma_start(
                    S_d[j][t0 : t0 + tsz, t0:NPj], sS[:]
                )

    nc.compile()
    return nc


def _get_nc(NPs_layout, group_sizes, u_dt, s16):
    key = (tuple(NPs_layout), tuple(group_sizes), u_dt, s16)
    if key not in _nc_cache:
        _nc_cache[key] = _build(NPs_layout, group_sizes, u_dt, s16)
    return _nc_cache[key]


def _round_fp32r(x, keep=11):
    """Round fp32 to fp32r (round-to-nearest-even at `keep` mantissa bits).
    Matches the TRN2 DVE fp32->fp32r cast bit-exactly (verified on HW)."""
    bits = x.view(np.uint32).astype(np.uint64)
    drop = 23 - keep
    half_ = np.uint64(1 << (drop - 1))
    one = np.uint64(1)
    lsb_mask = np.uint64((1 << drop) - 1)
    rounded = (bits + half_ - one + ((bits >> np.uint64(drop)) & one)) & ~lsb_mask
    return rounded.astype(np.uint32).view(np.float32)


def kernel(hidden_states, attention_mask, special_tokens_mask, W, b):
    hs = np.asarray(hidden_states, dtype=np.float32)
    am = np.asarray(attention_mask)
    sm = np.asarray(special_tokens_mask)
    Wm = np.ascontiguousarray(np.asarray(W, dtype=np.float32))
    bias = float(np.asarray(b).reshape(-1)[0])

    B, L, Hdim = hs.shape
    assert Hdim == H and B == N_CORES * SPB, (B, L, Hdim)

    m = (am == 1) & (sm == 0)
    idxs = [np.nonzero(m[i])[0] for i in range(B)]
    ns = np.array([len(ix) for ix in idxs])

    out = np.zeros((B, MAX_PAIRS), np.float32)
    if int(ns.max()) < 2:
        return out

    prec = os.environ.get("CONTACT_KERNEL_PRECISION", "balanced")
    u_dt, s16 = {
        "fast": ("fp16", True),
        "balanced": ("fp16", False),
        "accurate": ("fp32r", False),
        "exact": ("fp32", False),
    }[prec]

    # assign samples to (slot, core) by descending count; slot width = max
    # over its 8 cores so padding waste stays small
    order = np.argsort(-ns, kind="stable")
    # slot width = max count over its 8 cores, rounded UP to EVEN: PE
    # matmuls require even moving widths (odd widths fail the fp32r ISA
    # check and fault on hardware for fp16)
    NPs = [int(ns[order[8 * j]]) + (int(ns[order[8 * j]]) & 1) for j in range(SPB)]

    groups = _make_groups(NPs, u_dt)
    layout = [s for g in groups for s in g]  # packed column order of slots
    group_sizes = [len(g) for g in groups]
    NPs_layout = [NPs[s] for s in layout]
    offs = np.concatenate([[0], np.cumsum(NPs_layout)]).astype(int)

    nc = _get_nc(NPs_layout, group_sizes, u_dt, s16)

    if u_dt == "fp16":
        W_send = Wm.astype(np.float16)
        W_send[np.abs(Wm) < 6.2e-5] = np.float16(0)  # flush subnormals
    elif u_dt == "fp32r":
        W_send = _round_fp32r(Wm)
    else:
        W_send = Wm
    Wt = int(offs[-1])
    in_maps = []
    sample_at = {}
    for c in range(N_CORES):
        hT = np.zeros((H, Wt), np.float32)
        for p, s in enumerate(layout):
            bi = int(order[8 * s + c])
            sample_at[(c, p)] = bi
            ix = idxs[bi]
            if len(ix):
                hT[:, offs[p] : offs[p] + len(ix)] = hs[bi].T[:, ix]
        in_maps.append({"hT": hT, "W": W_send})

    res = run_bass_kernel_spmd(nc, in_maps, core_ids=list(range(N_CORES)))

    tri_cache: dict = {}
    for c in range(N_CORES):
        for p in range(SPB):
            bi = sample_at[(c, p)]
            n = int(ns[bi])
            if n < 2:
                continue
            S = res.results[c][f"S{p}"]
            if n not in tri_cache:
                tri_cache[n] = np.triu_indices(n, 1)
            iu, ju = tri_cache[n]
            out[bi, : iu.size] = S[iu, ju] + bias
    return out


# revision 21
# speedup vs baseline: 1.6563x; 1.1216x over previous
"""Trainium2 Bass kernel for nn_ContactMapHead.

Reference computation (per sample b):
  m = (attention_mask==1) & (special_tokens_mask==0)     # valid positions
  S[t,s] = h_t^T W h_s + bias                            # bilinear scores
  out[b, :] = strict-upper-triangle (row-major) of S restricted to the
              compacted valid positions, scattered into a MAX_PAIRS buffer
              (rest zeros).

Strategy (data-parallel over batch across 8 NeuronCores):
  - Host: compute valid index lists; gather + transpose each sample's valid
    rows into a packed hT buffer (H x Wt per core, zero padded to per-slot
    widths shared across cores so one SPMD NEFF serves all 8 cores). Index
    bookkeeping only — all tensor FLOPs run on device.
  - Device (per core, 4 samples): u^T = W^T @ hT, then per-sample
    S = u^T.T @ hT. Ships the per-sample S matrices back.
  - Host: out[b, :npairs] = S[triu] + bias; remainder stays zero.

Precision modes (CONTACT_KERNEL_PRECISION):
  fast:     u-step fp16, S-step fp16   (~5e-4 rel-absmax)
  balanced: u-step fp16, S-step fp32   (~2.5e-4)  [default]
  accurate: u-step fp32r, S-step fp32  (~1.3e-4)
  exact:    everything fp32            (~1e-6, ~3x slower)

Samples are assigned to (core, slot) by descending valid-count so per-slot
padded widths (max over cores) waste little compute. The NEFF is compiled
for the actual widths of the given inputs (cached per shape).
"""

import os

import numpy as np

import concourse.bass as bass  # noqa: F401
import concourse.tile as tile
from concourse import bacc, mybir
from concourse.bass_utils import run_bass_kernel_spmd

MAX_PAIRS = 130816
N_CORES = 8
H = 1024
KT = H // 128  # k-tiles along the contraction dim
SPB = 4  # samples (slots) per core

_nc_cache: dict = {}
LAST_EXEC_NS = None  # filled per call when NTFF tracing is available


def _make_groups(NPs, u_dt):
    """Partition slots into column groups, each of total width <= 512 (PSUM
    bank / moving-operand limit), minimizing modeled u-step cost. fp32r runs
    4 cyc/row below width 256; fp16/fp32 cost is width-proportional. Brute
    forces all set partitions of the (at most 4) slots."""
    n = len(NPs)

    def partitions(items):
        if not items:
            yield []
            return
        first, rest = items[0], items[1:]
        for part in partitions(rest):
            for i in range(len(part)):
                yield part[:i] + [[first] + part[i]] + part[i + 1 :]
            yield [[first]] + part

    def gcost(w):
        if u_dt == "fp32":
            return w * 4.0
        if u_dt == "fp32r":
            return w * (4.0 if w < 256 else 1.0)
        # fp16: per chain-step cost ~ max(weight-load ~107ns, stream w/2.4GHz)
        # (LDWEIGHTS pipelines against the previous matmul on HW)
        return max(107.0, w * 0.4167) / 0.4167

    best, best_cost = None, None
    for part in partitions(list(range(n))):
        cost = 0.0
        ok = True
        for g in part:
            w = sum(NPs[s] for s in g)
            if w > 512:
                ok = False
                break
            cost += gcost(w)
        if ok and (best_cost is None or (cost, len(part)) < best_cost):
            best_cost = (cost, len(part))
            best = part
    assert best is not None
    return [sorted(g) for g in best]


def _build(NPs_layout, group_sizes, u_dt, s16, loop_iters=None):
    """Build + compile the per-core NEFF.

    NPs_layout: per-slot padded widths in packed column order (groups are
    contiguous runs of slots given by group_sizes).
    loop_iters: benchmark mode — repeat the whole body N times on-device so
    wall-clock deltas between two iteration counts give per-iteration HW
    time through the axon tunnel (no NTFF profiling available there)."""
    f32 = mybir.dt.float32
    mm_dt = {"fp16": mybir.dt.float16, "fp32r": mybir.dt.float32r, "fp32": f32}[u_dt]
    s_dt = mybir.dt.float16 if s16 else f32
    Wt = sum(NPs_layout)
    offs = np.concatenate([[0], np.cumsum(NPs_layout)]).astype(int)
    granges = []
    s = 0
    for gs in group_sizes:
        granges.append((int(offs[s]), int(offs[s + gs])))
        s += gs
    n_groups = len(granges)

    nc = bacc.Bacc(
        "TRN2",
        target_bir_lowering=False,
        debug=False,
        enable_asserts=False,
        num_devices=N_CORES,
    )
    hT_d = nc.dram_tensor("hT", [H, Wt], f32, kind="ExternalInput").ap()
    W_d = nc.dram_tensor("W", [H, H], mm_dt, kind="ExternalInput").ap()
    S_d = [
        nc.dram_tensor(f"S{j}", [NPs_layout[j], NPs_layout[j]], f32, kind="ExternalOutput").ap()
        for j in range(SPB)
    ]

    out_engines = [nc.sync, nc.scalar]

    import contextlib

    with tile.TileContext(nc) as tc:
        with (
            tc.For_i(0, loop_iters, 1, hint_engines=(mybir.EngineType.PE,))
            if loop_iters
            else contextlib.nullcontext(),
            tc.tile_pool(name="wpool", bufs=1) as wpool,
            tc.tile_pool(name="hpool", bufs=1) as hpool,
            tc.tile_pool(name="upool", bufs=1) as upool,
            tc.tile_pool(name="spool", bufs=4) as spool,
            tc.tile_pool(name="ps", bufs=8, space="PSUM") as ps,
        ):
            # ---- PE warmup: keep HAM busy while input DMAs land ----
            # (plain fp32 warmup tiles: memset of float32r is illegal ISA)
            wz_a = wpool.tile([128, 16], f32, name="wz_a")
            nc.gpsimd.memset(wz_a[:], 0.0)
            wz_b = wpool.tile([128, 64], f32, name="wz_b")
            nc.gpsimd.memset(wz_b[:], 0.0)
            pwarm = ps.tile([16, 64], f32, tag="ps", name="pwarm")
            for _ in range(32):
                nc.tensor.matmul(pwarm[:], wz_a[:], wz_b[:], start=True, stop=True)
            # warm the ACT Copy table so the first real scalar.copy is cheap
            wz_c = wpool.tile([16, 64], f32, name="wz_c")
            nc.scalar.copy(wz_c[:], pwarm[:])

            # ---- W k-tiles (pre-rounded on host; SP-triggered DMA) ----
            W_mm = []
            for kk in range(KT):
                wr = wpool.tile([128, H], mm_dt, name=f"wr{kk}")
                nc.sync.dma_start(wr[:], W_d[128 * kk : 128 * (kk + 1), :])
                W_mm.append(wr)

            # ---- hT k-tiles (ACT-triggered DMA; DVE cast for the u-step) ----
            h32 = []
            h_mm = []
            for kk in range(KT):
                h = hpool.tile([128, Wt], f32, name=f"h32_{kk}")
                nc.scalar.dma_start(h[:], hT_d[128 * kk : 128 * (kk + 1), :])
                h32.append(h)
                if u_dt == "fp32":
                    h_mm.append(h)
                else:
                    hr = hpool.tile([128, Wt], mm_dt, name=f"hr{kk}")
                    nc.vector.tensor_copy(hr[:], h[:])
                    h_mm.append(hr)

            # S-step operand tiles (fp16 S reads the cast h; fp32 S reads h32)
            h_s = h_mm if s16 else h32

            # ---- u-step: u^T[k',c] = sum_k W[k,k'] hT[k,c] ----
            # kk-outer accumulation so the PE consumes k-tiles as they
            # arrive. Live PSUM chains (n_groups x kkp_wave) bounded by the
            # 8 PSUM banks. For the slower fp32r/fp32 paths (longer DMA
            # window), the contraction is additionally split into kk-waves
            # with SBUF accumulation so every kkp-chain makes progress while
            # early k-tiles arrive.
            u_sb = [upool.tile([128, Wt], s_dt, name=f"u_{kk}") for kk in range(KT)]
            kkp_wave = max(1, 8 // n_groups)
            kk_wave = KT if (n_groups == 1 or u_dt == "fp16") else KT // 2
            for w0 in range(0, KT, kk_wave):
                kks = range(w0, min(w0 + kk_wave, KT))
                for h0 in range(0, KT, kkp_wave):
                    kkps = range(h0, min(h0 + kkp_wave, KT))
                    chains = {}
                    for kk in kks:
                        for gi, (c0, c1) in enumerate(granges):
                            for kkp in kkps:
                                if kk == kks[0]:
                                    chains[(gi, kkp)] = ps.tile(
                                        [128, c1 - c0],
                                        f32,
                                        tag="ps",
                                        name=f"ups{gi}_{kkp}_{w0}",
                                    )
                                nc.tensor.matmul(
                                    chains[(gi, kkp)][:],
                                    W_mm[kk][:, 128 * kkp : 128 * (kkp + 1)],
                                    h_mm[kk][:, c0:c1],
                                    start=(kk == kks[0]),
                                    stop=(kk == kks[-1]),
                                )
                    for gi, (c0, c1) in enumerate(granges):
                        for kkp in kkps:
                            dst = u_sb[kkp][:, c0:c1]
                            src = chains[(gi, kkp)][:]
                            if w0 == 0:
                                if kkp % 2 == 0:
                                    nc.vector.tensor_copy(dst, src)
                                else:
                                    nc.scalar.copy(dst, src)
                            else:
                                # elementwise tensor+tensor add is DVE-only
                                nc.vector.tensor_add(dst, dst, src)

            # ---- S-step: S[t,s] = sum_k' u^T[k',t] hT[k',s] ----
            # Largest chunks first so the kernel tail ends on the smallest
            # chunk's copy + DMA.
            # Chunks stream cols [t0, NPj) — the diagonal column is computed
            # and ignored by the host so the moving width (NPj - t0) stays
            # EVEN (PE matmuls fault on odd moving widths; NPj is even).
            s_chunks = []
            for j in range(SPB):
                NPj = NPs_layout[j]
                for t0 in range(0, NPj, 128):
                    tsz = min(128, NPj - t0)
                    nsz = NPj - t0
                    if nsz > 1:
                        s_chunks.append((tsz * nsz, j, t0, tsz, nsz))
            s_chunks.sort(reverse=True)
            for ci, (_, j, t0, tsz, nsz) in enumerate(s_chunks):
                o = int(offs[j])
                NPj = NPs_layout[j]
                pS = ps.tile([tsz, nsz], f32, tag="ps", name=f"sps{j}_{t0}")
                for kk in range(KT):
                    nc.tensor.matmul(
                        pS[:],
                        u_sb[kk][:, o + t0 : o + t0 + tsz],
                        h_s[kk][:, o + t0 : o + NPj],
                        start=(kk == 0),
                        stop=(kk == KT - 1),
                    )
                sS = spool.tile([tsz, nsz], f32, tag="s_sb")
                if ci % 2 == 0:
                    nc.vector.tensor_copy(sS[:], pS[:])
                else:
                    nc.scalar.copy(sS[:], pS[:])
                out_engines[ci % len(out_engines)].dma_start(
                    S_d[j][t0 : t0 + tsz, t0:NPj], sS[:]
                )

    nc.compile()
    return nc


def _get_nc(NPs_layout, group_sizes, u_dt, s16):
    key = (tuple(NPs_layout), tuple(group_sizes), u_dt, s16)
    if key not in _nc_cache:
        _nc_cache[key] = _build(NPs_layout, group_sizes, u_dt, s16)
    return _nc_cache[key]


def _round_fp32r(x, keep=11):
    """Round fp32 to fp32r (round-to-nearest-even at `keep` mantissa bits).
    Matches the TRN2 DVE fp32->fp32r cast bit-exactly (verified on HW)."""
    bits = x.view(np.uint32).astype(np.uint64)
    drop = 23 - keep
    half_ = np.uint64(1 << (drop - 1))
    one = np.uint64(1)
    lsb_mask = np.uint64((1 << drop) - 1)
    rounded = (bits + half_ - one + ((bits >> np.uint64(drop)) & one)) & ~lsb_mask
    return rounded.astype(np.uint32).view(np.float32)


def kernel(hidden_states, attention_mask, special_tokens_mask, W, b):
    hs = np.asarray(hidden_states, dtype=np.float32)
    am = np.asarray(attention_mask)
    sm = np.asarray(special_tokens_mask)
    Wm = np.ascontiguousarray(np.asarray(W, dtype=np.float32))
    bias = float(np.asarray(b).reshape(-1)[0])

    B, L, Hdim = hs.shape
    assert Hdim == H and B == N_CORES * SPB, (B, L, Hdim)

    m = (am == 1) & (sm == 0)
    idxs = [np.nonzero(m[i])[0] for i in range(B)]
    ns = np.array([len(ix) for ix in idxs])

    out = np.zeros((B, MAX_PAIRS), np.float32)
    if int(ns.max()) < 2:
        return out

    prec = os.environ.get("CONTACT_KERNEL_PRECISION", "balanced")
    u_dt, s16 = {
        "fast": ("fp16", True),
        "balanced": ("fp16", False),
        "accurate": ("fp32r", False),
        "exact": ("fp32", False),
    }[prec]

    # assign samples to (slot, core) by descending count; slot width = max
    # over its 8 cores so padding waste stays small
    order = np.argsort(-ns, kind="stable")
    # slot width = max count over its 8 cores, rounded UP to EVEN: PE
    # matmuls require even moving widths (odd widths fail the fp32r ISA
    # check and fault on hardware for fp16)
    NPs = [int(ns[order[8 * j]]) + (int(ns[order[8 * j]]) & 1) for j in range(SPB)]

    groups = _make_groups(NPs, u_dt)
    layout = [s for g in groups for s in g]  # packed column order of slots
    group_sizes = [len(g) for g in groups]
    NPs_layout = [NPs[s] for s in layout]
    offs = np.concatenate([[0], np.cumsum(NPs_layout)]).astype(int)

    nc = _get_nc(NPs_layout, group_sizes, u_dt, s16)

    if u_dt == "fp16":
        W_send = Wm.astype(np.float16)
        W_send[np.abs(Wm) < 6.2e-5] = np.float16(0)  # flush subnormals
    elif u_dt == "fp32r":
        W_send = _round_fp32r(Wm)
    else:
        W_send = Wm
    Wt = int(offs[-1])
    in_maps = []
    sample_at = {}
    for c in range(N_CORES):
        hT = np.zeros((H, Wt), np.float32)
        for p, s in enumerate(layout):
            bi = int(order[8 * s + c])
            sample_at[(c, p)] = bi
            ix = idxs[bi]
            if len(ix):
                hT[:, offs[p] : offs[p] + len(ix)] = hs[bi].T[:, ix]
        in_maps.append({"hT": hT, "W": W_send})

    trace = os.environ.get("CONTACT_KERNEL_TRACE", "0") == "1"
    res = run_bass_kernel_spmd(
        nc, in_maps, core_ids=list(range(N_CORES)), trace=trace
    )
    global LAST_EXEC_NS
    LAST_EXEC_NS = res.exec_time_ns

    tri_cache: dict = {}
    for c in range(N_CORES):
        for p in range(SPB):
            bi = sample_at[(c, p)]
            n = int(ns[bi])
            if n < 2:
                continue
            S = res.results[c][f"S{p}"]
            if n not in tri_cache:
                tri_cache[n] = np.triu_indices(n, 1)
            iu, ju = tri_cache[n]
            out[bi, : iu.size] = S[iu, ju] + bias
    return out
